# revision 27
# baseline (speedup 1.0000x reference)
"""Causal self-attention (B=2, T=2048, C=1024, H=16) on 8 TRN2 NeuronCores.

Sharding: core c -> batch b = c//4, head group hg = c%4 (4 heads/core).
Each core computes QKV for its 4 heads (column-parallel), causal attention,
and a row-parallel partial output projection [T, C]. The host sums the 4
partials per batch and adds the analytically-folded biases.

Device layouts (chosen so no on-chip transposes are ever needed):
  xt   [C=1024, T=2048] bf16   x[b] transposed (host-prepped)
  Q^T  [128, pair, T]   bf16   head pair packed on partitions (0-63 / 64-127)
  K^T  same
  vaug [128, tj, 4*66]  bf16   per head: col0 = ones, cols1-64 = V[tj block]
  S^T  [k=128, q<=512]  psum   row-packed K=64 matmuls, 2 heads concurrent
  P^T = exp(S^T)        bf16   (no max subtraction; scores are ~N(0,1))
  O^T  [65, 512] psum:  row0 = softmax denominator l, rows 1-64 = (P@V)^T
  yt   [128(h,d), T]    bf16   normalized attention output, feeds proj lhsT
"""

import sys

if "/opt/trn_rl_repo" not in sys.path:
    sys.path.insert(0, "/opt/trn_rl_repo")

import numpy as np
import ml_dtypes
from contextlib import ExitStack

import concourse.bass as bass
import concourse.mybir as mybir
import concourse.tile as tile
from concourse import bacc, bass_utils
from concourse.bass import ds, ts



BF = mybir.dt.bfloat16
F32 = mybir.dt.float32

B, T, C = 2, 2048, 1024
H, DK = 16, 64
P = 128
KC = C // P          # 8 contraction chunks over C
NTG = T // 512       # 4 t-groups of 512
NTJ = T // 128       # 16 t-chunks of 128
HPC = 4              # heads per core
VS = 66              # vaug per-head stride (col0 ones, 1-64 V, 65 pad)

NEG = -30000.0

# module-level knobs for test harness
TRACE = False
TRACE_KWARGS = {}
LAST_RESULTS = None


def _emit(ctx, tc, aps):
    nc = tc.nc
    xt, wq, wk, wv, bq, bk, wp, mask, out = (
        aps["xt"], aps["wq"], aps["wk"], aps["wv"], aps["bq"], aps["bk"],
        aps["wp"], aps["mask"], aps["out"],
    )

    consts = ctx.enter_context(tc.tile_pool(name="consts", bufs=1))
    bigs = ctx.enter_context(tc.tile_pool(name="bigs", bufs=1))
    temps = ctx.enter_context(tc.tile_pool(name="temps", bufs=4))
    ppool = ctx.enter_context(tc.tile_pool(name="ppool", bufs=4))
    psum = ctx.enter_context(tc.tile_pool(name="psum", bufs=1, space="PSUM"))
    dpool = ctx.enter_context(tc.tile_pool(name="dpool", bufs=2, space="DRAM"))

    # ---- load inputs to SBUF (weights first — the first matmuls need them;
    # xts split per t-group, alternating DMA engines to parallelize) ----
    wqs = consts.tile([P, KC, 2 * P], BF)
    nc.sync.dma_start(out=wqs, in_=wq.rearrange("(k p) n -> p k n", p=P))
    wks = consts.tile([P, KC, 2 * P], BF)
    nc.gpsimd.dma_start(out=wks, in_=wk.rearrange("(k p) n -> p k n", p=P))
    bqs = consts.tile([P, 2], F32)
    nc.sync.dma_start(out=bqs, in_=bq.rearrange("(m p) -> p m", p=P))
    bks = consts.tile([P, 2], F32)
    nc.sync.dma_start(out=bks, in_=bk.rearrange("(m p) -> p m", p=P))
    maskt = consts.tile([P, P], F32)
    nc.sync.dma_start(out=maskt, in_=mask)

    xts = bigs.tile([P, KC, T], BF)
    xtr = xt.rearrange("(k p) t -> p k t", p=P)
    for tg in range(NTG):
        eng = nc.sync if tg % 2 == 0 else nc.gpsimd
        eng.dma_start(out=xts[:, :, ts(tg, 512)], in_=xtr[:, :, ts(tg, 512)])

    wvs = consts.tile([P, KC, 2 * P], BF)
    nc.gpsimd.dma_start(out=wvs, in_=wv.rearrange("(k p) n -> p k n", p=P))
    wps = consts.tile([P, 2, C], BF)
    nc.sync.dma_start(out=wps, in_=wp.rearrange("(k p) n -> p k n", p=P))

    # ---- Q^T / K^T: [128(d pair-packed), pair, T] ----
    qt = bigs.tile([P, 2, T], BF)
    kt = bigs.tile([P, 2, T], BF)

    def emit_qk(m, tg):
        for wsrc, bsrc, dst in ((wqs, bqs, qt), (wks, bks, kt)):
            pqk = psum.tile([P, 512], F32, tag="mm", bufs=2, name="pqk")
            for k in range(KC):
                nc.tensor.matmul(
                    pqk,
                    lhsT=wsrc[:, k, ts(m, P)],
                    rhs=xts[:, k, ts(tg, 512)],
                    start=(k == 0),
                    stop=(k == KC - 1),
                )
            nc.vector.tensor_add(
                out=dst[:, m, ts(tg, 512)],
                in0=pqk,
                in1=bsrc[:, m : m + 1].to_broadcast([P, 512]),
            )

    # ---- V -> vaug [128, tj, 4*66] (col DK = ones) ----
    vaug = bigs.tile([P, NTJ, HPC * VS], BF)
    vaug4 = vaug.rearrange("p t (h c) -> p t h c", c=VS)

    def emit_v(g):
        for tj in range(4 * g, 4 * g + 4):
            pv = psum.tile([P, 512], F32, tag="mm", bufs=2, name="pv")
            for k in range(KC):
                nc.tensor.matmul(
                    pv[:, : 2 * P],
                    lhsT=xts[:, k, ts(tj, P)],
                    rhs=wvs[:, k, :],
                    start=(k == 0),
                    stop=(k == KC - 1),
                )
            nc.vector.tensor_copy(
                out=vaug4[:, tj, :, 0:DK],
                in_=pv[:, : 2 * P].rearrange("p (h d) -> p h d", d=DK),
            )

    # ---- attention ----
    yts = [bigs.tile([P, T], BF, name=f"yt{m}") for m in range(2)]

    def emit_attn(m, g):
        po = [
            psum.tile([DK + 1, 512], F32, tag=f"o{h}", bufs=1, name=f"po{h}")
            for h in range(2)
        ]
        njc = 4 * g + 4
        for j in range(njc):
            jrel = j - 4 * g
            band = jrel >= 0
            ncols = 512 - 128 * jrel if band else 512
            qoff = g * 512 + (128 * jrel if band else 0)
            pss = []
            for h in range(2):
                ps = psum.tile([P, 512], F32, tag=f"s{h}", bufs=2, name=f"ps{h}")
                nc.tensor.matmul(
                    ps[:, :ncols],
                    lhsT=kt[h * DK : (h + 1) * DK, m, ts(j, P)],
                    rhs=qt[h * DK : (h + 1) * DK, m, ds(qoff, ncols)],
                    start=True,
                    stop=True,
                    tile_position=(h * DK, 0),
                )
                pss.append(ps)
            if band:
                for h in range(2):
                    nc.vector.tensor_add(
                        out=pss[h][:, :P], in0=pss[h][:, :P], in1=maskt
                    )
            for h in range(2):
                pt = ppool.tile([P, 512], BF, tag=f"p{h}", name=f"pt{h}")
                nc.scalar.activation(
                    pt[:, :ncols],
                    pss[h][:, :ncols],
                    mybir.ActivationFunctionType.Exp,
                )
                co = 128 * jrel if band else 0
                nc.tensor.matmul(
                    po[h][:, co : co + ncols],
                    lhsT=vaug4[:, j, 2 * m + h, : DK + 1],
                    rhs=pt[:, :ncols],
                    start=(j == 0),
                    stop=(j == njc - 1),
                    skip_group_check=True,
                )
        # finalize: copy O^T off PSUM fast, then normalize rows 0-63 by the
        # broadcast exp-sum (row 64) and place into yt
        for h in range(2):
            oc = temps.tile([P, 512], F32, tag="oc", name="oc")
            nc.vector.tensor_copy(out=oc[: DK + 1, :], in_=po[h])
            dscr = dpool.tile([512], F32, tag="dscr", name="dscr")
            nc.sync.dma_start(out=dscr, in_=oc[DK : DK + 1, :])
            rbl = temps.tile([P, 512], F32, tag="rbl", name="rbl")
            nc.gpsimd.dma_start(
                out=rbl[:DK, :],
                in_=bass.AP(
                    tensor=dscr.tensor,
                    offset=dscr.offset,
                    ap=[[0, DK]] + list(dscr.ap),
                ),
            )
            rb = temps.tile([P, 512], F32, tag="rb", name="rb")
            nc.vector.reciprocal_approx_fast(out=rb[:DK, :], in_=rbl[:DK, :])
            stg = temps.tile([P, 512], BF, tag="stg", name="stg")
            nc.vector.tensor_mul(
                out=stg[:DK, :],
                in0=oc[:DK, :],
                in1=rb[:DK, :],
            )
            nc.sync.dma_start(
                out=yts[m][h * DK : (h + 1) * DK, ts(g, 512)],
                in_=stg[:DK, :],
            )

    # ---- output projection: partial [T, C] for one t-group of 4 chunks ----
    def emit_proj(g):
        for tj in range(4 * g, 4 * g + 4):
            pps = [
                psum.tile([P, 512], F32, tag="mm", bufs=2, name=f"pp{n}")
                for n in range(2)
            ]
            for kc in range(2):
                for n in range(2):
                    nc.tensor.matmul(
                        pps[n],
                        lhsT=yts[kc][:, ts(tj, P)],
                        rhs=wps[:, kc, ts(n, 512)],
                        start=(kc == 0),
                        stop=(kc == 1),
                    )
            for n in range(2):
                ostg = temps.tile([P, 512], F32, tag="ostg", name="ostg")
                nc.vector.tensor_copy(out=ostg, in_=pps[n])
                nc.sync.dma_start(out=out[ts(tj, P), ts(n, 512)], in_=ostg)

    # ---- schedule: pipeline by q-group, weaving PE-dense QKV/proj work
    # between ACT-gated attention so both engine queues stay fed. Group
    # order [1,2,3,0] puts the smallest attention group (g=0) last so the
    # kernel tail is short. QKV/V tiles are emitted incrementally just
    # before the first group that needs them. ----
    nc.vector.memset(vaug4[:, :, :, DK : DK + 1], 1.0)
    order = [1, 2, 3, 0]
    qk_done = [0, 0]  # per pair: number of t-groups emitted
    v_done = 0
    proj_queue = []
    for g in order:
        need = g + 1
        while v_done < need:
            if qk_done[0] < v_done + 1:
                emit_qk(0, qk_done[0])
                qk_done[0] += 1
            emit_v(v_done)
            v_done += 1
        while qk_done[0] < need:
            emit_qk(0, qk_done[0])
            qk_done[0] += 1
        emit_attn(0, g)
        while qk_done[1] < need:
            emit_qk(1, qk_done[1])
            qk_done[1] += 1
        emit_attn(1, g)
        proj_queue.append(g)
        if len(proj_queue) > 1:
            emit_proj(proj_queue.pop(0))
    for g in proj_queue:
        emit_proj(g)


_NC_CACHE = None


def build():
    global _NC_CACHE
    if _NC_CACHE is not None:
        return _NC_CACHE
    nc = bacc.Bacc("TRN2", target_bir_lowering=False, debug=False, num_devices=8)
    aps = {
        "xt": nc.dram_tensor("xt", [C, T], BF, kind="ExternalInput").ap(),
        "wq": nc.dram_tensor("wq", [C, 2 * P], BF, kind="ExternalInput").ap(),
        "wk": nc.dram_tensor("wk", [C, 2 * P], BF, kind="ExternalInput").ap(),
        "wv": nc.dram_tensor("wv", [C, 2 * P], BF, kind="ExternalInput").ap(),
        "bq": nc.dram_tensor("bq", [2 * P], F32, kind="ExternalInput").ap(),
        "bk": nc.dram_tensor("bk", [2 * P], F32, kind="ExternalInput").ap(),
        "wp": nc.dram_tensor("wp", [2 * P, C], BF, kind="ExternalInput").ap(),
        "mask": nc.dram_tensor("mask", [P, P], F32, kind="ExternalInput").ap(),
        "out": nc.dram_tensor("out", [T, C], F32, kind="ExternalOutput").ap(),
    }
    with tile.TileContext(nc) as tc:
        with ExitStack() as ctx:
            _emit(ctx, tc, aps)
    nc.compile()
    _NC_CACHE = nc
    return nc


def make_in_maps(x, Wqkv, bqkv, Wproj):
    """Host-side sharding/layout prep. Returns per-core input dicts."""
    bf = ml_dtypes.bfloat16
    scale = np.float32(1.0 / np.sqrt(DK))
    maskv = np.where(
        np.arange(P)[None, :] >= np.arange(P)[:, None], 0.0, NEG
    ).astype(np.float32)
    xts = [np.ascontiguousarray(x[b].T).astype(bf) for b in range(B)]
    in_maps = []
    for c in range(8):
        b, hg = divmod(c, 4)
        lo = hg * HPC * DK
        sl = slice(lo, lo + HPC * DK)
        in_maps.append(
            {
                "xt": xts[b],
                "wq": np.ascontiguousarray(Wqkv[:, 0 * C :][:, sl] * scale).astype(bf),
                "wk": np.ascontiguousarray(Wqkv[:, 1 * C :][:, sl]).astype(bf),
                "wv": np.ascontiguousarray(Wqkv[:, 2 * C :][:, sl]).astype(bf),
                "bq": np.ascontiguousarray(bqkv[0 * C :][sl] * scale).astype(np.float32),
                "bk": np.ascontiguousarray(bqkv[1 * C :][sl]).astype(np.float32),
                "wp": np.ascontiguousarray(Wproj[sl, :]).astype(bf),
                "mask": maskv,
            }
        )
    return in_maps


def gather(outs, bqkv, Wproj, bproj):
    """Sum per-core partials per batch; fold V-bias + proj-bias analytically."""
    bv = bqkv[2 * C :].astype(np.float32)
    bp_eff = (bproj.astype(np.float32) + bv @ Wproj.astype(np.float32)).astype(
        np.float32
    )
    y = np.empty((B, T, C), np.float32)
    for b in range(B):
        acc = outs[b * 4 + 0].astype(np.float32).copy()
        for hg in range(1, 4):
            acc += outs[b * 4 + hg]
        y[b] = acc + bp_eff[None, :]
    return y


def kernel(x, Wqkv, bqkv, Wproj, bproj):
    global LAST_RESULTS
    x = np.asarray(x, dtype=np.float32)
    Wqkv = np.asarray(Wqkv, dtype=np.float32)
    bqkv = np.asarray(bqkv, dtype=np.float32)
    Wproj = np.asarray(Wproj, dtype=np.float32)
    bproj = np.asarray(bproj, dtype=np.float32)

    nc = build()
    in_maps = make_in_maps(x, Wqkv, bqkv, Wproj)
    try:
        res = bass_utils.run_bass_kernel_spmd(
            nc,
            in_maps,
            core_ids=list(range(8)),
            trace=TRACE,
            **TRACE_KWARGS,
        )
    except Exception:
        if not TRACE:
            raise
        import traceback

        traceback.print_exc()
        print("traced run failed; retrying without trace", file=sys.stderr)
        res = bass_utils.run_bass_kernel_spmd(nc, in_maps, core_ids=list(range(8)))
    LAST_RESULTS = res
    outs = [res.results[c]["out"] for c in range(8)]
    return gather(outs, bqkv, Wproj, bproj)


# revision 32
# speedup vs baseline: 1.0069x; 1.0069x over previous
"""Causal self-attention (B=2, T=2048, C=1024, H=16) on 8 TRN2 NeuronCores.

Sharding: core c -> batch b = c//4, head group hg = c%4 (4 heads/core).
Each core computes QKV for its 4 heads (column-parallel), causal attention,
and a row-parallel partial output projection [T, C]. The host sums the 4
partials per batch and adds the analytically-folded biases.

Device layouts (chosen so no on-chip transposes are ever needed):
  xt   [C=1024, T=2048] bf16   x[b] transposed (host-prepped)
  Q^T  [128, pair, T]   bf16   head pair packed on partitions (0-63 / 64-127)
  K^T  same
  vaug [128, tj, 4*66]  bf16   per head: col0 = ones, cols1-64 = V[tj block]
  S^T  [k=128, q<=512]  psum   row-packed K=64 matmuls, 2 heads concurrent
  P^T = exp(S^T)        bf16   (no max subtraction; scores are ~N(0,1))
  O^T  [65, 512] psum:  row0 = softmax denominator l, rows 1-64 = (P@V)^T
  yt   [128(h,d), T]    bf16   normalized attention output, feeds proj lhsT
"""

import sys

if "/opt/trn_rl_repo" not in sys.path:
    sys.path.insert(0, "/opt/trn_rl_repo")

import numpy as np
import ml_dtypes
from contextlib import ExitStack

import concourse.bass as bass
import concourse.mybir as mybir
import concourse.tile as tile
from concourse import bacc, bass_utils
from concourse.bass import ds, ts



BF = mybir.dt.bfloat16
F32 = mybir.dt.float32

B, T, C = 2, 2048, 1024
H, DK = 16, 64
P = 128
KC = C // P          # 8 contraction chunks over C
NTG = T // 512       # 4 t-groups of 512
NTJ = T // 128       # 16 t-chunks of 128
HPC = 4              # heads per core
VS = 66              # vaug per-head stride (col0 ones, 1-64 V, 65 pad)

NEG = -30000.0

# module-level knobs for test harness
TRACE = False
TRACE_KWARGS = {}
LAST_RESULTS = None


def _emit(ctx, tc, aps):
    nc = tc.nc
    xt, wq, wk, wv, bq, bk, wp, mask, out = (
        aps["xt"], aps["wq"], aps["wk"], aps["wv"], aps["bq"], aps["bk"],
        aps["wp"], aps["mask"], aps["out"],
    )

    consts = ctx.enter_context(tc.tile_pool(name="consts", bufs=1))
    bigs = ctx.enter_context(tc.tile_pool(name="bigs", bufs=1))
    temps = ctx.enter_context(tc.tile_pool(name="temps", bufs=4))
    ppool = ctx.enter_context(tc.tile_pool(name="ppool", bufs=4))
    psum = ctx.enter_context(tc.tile_pool(name="psum", bufs=1, space="PSUM"))
    dpool = ctx.enter_context(tc.tile_pool(name="dpool", bufs=2, space="DRAM"))

    # ---- load inputs to SBUF (weights first — the first matmuls need them;
    # xts split per t-group, alternating DMA engines to parallelize) ----
    wqs = consts.tile([P, KC, 2 * P], BF)
    nc.sync.dma_start(out=wqs, in_=wq.rearrange("(k p) n -> p k n", p=P))
    wks = consts.tile([P, KC, 2 * P], BF)
    nc.gpsimd.dma_start(out=wks, in_=wk.rearrange("(k p) n -> p k n", p=P))
    bqs = consts.tile([P, 2], F32)
    nc.sync.dma_start(out=bqs, in_=bq.rearrange("(m p) -> p m", p=P))
    bks = consts.tile([P, 2], F32)
    nc.sync.dma_start(out=bks, in_=bk.rearrange("(m p) -> p m", p=P))
    maskt = consts.tile([P, P], F32)
    nc.sync.dma_start(out=maskt, in_=mask)

    xts = bigs.tile([P, KC, T], BF)
    xtr = xt.rearrange("(k p) t -> p k t", p=P)
    for tg in range(NTG):
        eng = nc.sync if tg % 2 == 0 else nc.gpsimd
        eng.dma_start(out=xts[:, :, ts(tg, 512)], in_=xtr[:, :, ts(tg, 512)])

    wvs = consts.tile([P, KC, 2 * P], BF)
    nc.gpsimd.dma_start(out=wvs, in_=wv.rearrange("(k p) n -> p k n", p=P))
    wps = consts.tile([P, 2, C], BF)
    nc.sync.dma_start(out=wps, in_=wp.rearrange("(k p) n -> p k n", p=P))

    # ---- Q^T / K^T: [128(d pair-packed), pair, T] ----
    qt = bigs.tile([P, 2, T], BF)
    kt = bigs.tile([P, 2, T], BF)

    def emit_qk_pair(m, pg):
        # one LDWEIGHTS per (k, dst) feeds two N=512 matmuls (t-groups
        # 2*pg and 2*pg+1) — the second, identical LDW is deduped later
        tga, tgb = 2 * pg, 2 * pg + 1
        for wsrc, bsrc, dst in ((wqs, bqs, qt), (wks, bks, kt)):
            pq2 = [
                psum.tile([P, 512], F32, tag="mm", bufs=2, name=f"pq{i}")
                for i in range(2)
            ]
            for k in range(KC):
                for i, tg in enumerate((tga, tgb)):
                    nc.tensor.matmul(
                        pq2[i],
                        lhsT=wsrc[:, k, ts(m, P)],
                        rhs=xts[:, k, ts(tg, 512)],
                        start=(k == 0),
                        stop=(k == KC - 1),
                    )
            for i, tg in enumerate((tga, tgb)):
                nc.vector.tensor_add(
                    out=dst[:, m, ts(tg, 512)],
                    in0=pq2[i],
                    in1=bsrc[:, m : m + 1].to_broadcast([P, 512]),
                )

    # ---- V -> vaug [128, tj, 4*66] (col DK = ones) ----
    vaug = bigs.tile([P, NTJ, HPC * VS], BF)
    vaug4 = vaug.rearrange("p t (h c) -> p t h c", c=VS)

    def emit_v(g):
        for tj in range(4 * g, 4 * g + 4):
            pv = psum.tile([P, 512], F32, tag="mm", bufs=2, name="pv")
            for k in range(KC):
                nc.tensor.matmul(
                    pv[:, : 2 * P],
                    lhsT=xts[:, k, ts(tj, P)],
                    rhs=wvs[:, k, :],
                    start=(k == 0),
                    stop=(k == KC - 1),
                )
            nc.vector.tensor_copy(
                out=vaug4[:, tj, :, 0:DK],
                in_=pv[:, : 2 * P].rearrange("p (h d) -> p h d", d=DK),
            )

    # ---- attention ----
    yts = [bigs.tile([P, T], BF, name=f"yt{m}") for m in range(2)]

    def emit_attn(m, g):
        po = [
            psum.tile([DK + 1, 512], F32, tag=f"o{h}", bufs=1, name=f"po{h}")
            for h in range(2)
        ]
        njc = 4 * g + 4
        for j in range(njc):
            jrel = j - 4 * g
            band = jrel >= 0
            ncols = 512 - 128 * jrel if band else 512
            qoff = g * 512 + (128 * jrel if band else 0)
            pss = []
            for h in range(2):
                ps = psum.tile([P, 512], F32, tag=f"s{h}", bufs=2, name=f"ps{h}")
                nc.tensor.matmul(
                    ps[:, :ncols],
                    lhsT=kt[h * DK : (h + 1) * DK, m, ts(j, P)],
                    rhs=qt[h * DK : (h + 1) * DK, m, ds(qoff, ncols)],
                    start=True,
                    stop=True,
                    tile_position=(h * DK, 0),
                )
                pss.append(ps)
            if band:
                for h in range(2):
                    nc.vector.tensor_add(
                        out=pss[h][:, :P], in0=pss[h][:, :P], in1=maskt
                    )
            for h in range(2):
                pt = ppool.tile([P, 512], BF, tag=f"p{h}", name=f"pt{h}")
                nc.scalar.activation(
                    pt[:, :ncols],
                    pss[h][:, :ncols],
                    mybir.ActivationFunctionType.Exp,
                )
                co = 128 * jrel if band else 0
                nc.tensor.matmul(
                    po[h][:, co : co + ncols],
                    lhsT=vaug4[:, j, 2 * m + h, : DK + 1],
                    rhs=pt[:, :ncols],
                    start=(j == 0),
                    stop=(j == njc - 1),
                    skip_group_check=True,
                )
        # finalize: copy O^T off PSUM fast, then normalize rows 0-63 by the
        # broadcast exp-sum (row 64) and place into yt
        for h in range(2):
            oc = temps.tile([P, 512], F32, tag="oc", name="oc")
            nc.vector.tensor_copy(out=oc[: DK + 1, :], in_=po[h])
            dscr = dpool.tile([512], F32, tag="dscr", name="dscr")
            nc.sync.dma_start(out=dscr, in_=oc[DK : DK + 1, :])
            rbl = temps.tile([P, 512], F32, tag="rbl", name="rbl")
            nc.gpsimd.dma_start(
                out=rbl[:DK, :],
                in_=bass.AP(
                    tensor=dscr.tensor,
                    offset=dscr.offset,
                    ap=[[0, DK]] + list(dscr.ap),
                ),
            )
            rb = temps.tile([P, 512], F32, tag="rb", name="rb")
            nc.vector.reciprocal_approx_fast(out=rb[:DK, :], in_=rbl[:DK, :])
            stg = temps.tile([P, 512], BF, tag="stg", name="stg")
            nc.vector.tensor_mul(
                out=stg[:DK, :],
                in0=oc[:DK, :],
                in1=rb[:DK, :],
            )
            nc.sync.dma_start(
                out=yts[m][h * DK : (h + 1) * DK, ts(g, 512)],
                in_=stg[:DK, :],
            )

    # ---- output projection: partial [T, C] for one t-group of 4 chunks ----
    def emit_proj(g):
        for tj in range(4 * g, 4 * g + 4):
            pps = [
                psum.tile([P, 512], F32, tag="mm", bufs=2, name=f"pp{n}")
                for n in range(2)
            ]
            for kc in range(2):
                for n in range(2):
                    nc.tensor.matmul(
                        pps[n],
                        lhsT=yts[kc][:, ts(tj, P)],
                        rhs=wps[:, kc, ts(n, 512)],
                        start=(kc == 0),
                        stop=(kc == 1),
                    )
            for n in range(2):
                ostg = temps.tile([P, 512], F32, tag="ostg", name="ostg")
                nc.vector.tensor_copy(out=ostg, in_=pps[n])
                nc.sync.dma_start(out=out[ts(tj, P), ts(n, 512)], in_=ostg)

    # ---- schedule: pipeline by q-group, weaving PE-dense QKV/proj work
    # between ACT-gated attention so both engine queues stay fed. Group
    # order [1,2,3,0] puts the smallest attention group (g=0) last so the
    # kernel tail is short. QKV/V tiles are emitted incrementally just
    # before the first group that needs them. ----
    nc.vector.memset(vaug4[:, :, :, DK : DK + 1], 1.0)
    order = [1, 2, 3, 0]
    qk_done = [0, 0]  # per head-pair: number of t-group PAIRS emitted
    v_done = 0
    proj_queue = []
    for g in order:
        need_pg = g // 2 + 1
        need_v = g + 1
        while qk_done[0] < need_pg:
            emit_qk_pair(0, qk_done[0])
            qk_done[0] += 1
        while v_done < need_v:
            emit_v(v_done)
            v_done += 1
        emit_attn(0, g)
        while qk_done[1] < need_pg:
            emit_qk_pair(1, qk_done[1])
            qk_done[1] += 1
        emit_attn(1, g)
        proj_queue.append(g)
        if len(proj_queue) > 1:
            emit_proj(proj_queue.pop(0))
    for g in proj_queue:
        emit_proj(g)


def _dedupe_ldweights(nc):
    """Drop an InstLdweights when the immediately-preceding PE weight load in
    the scheduled stream is byte-identical (only matmuls in between — they
    don't disturb the stationary operand). Saves ~100ns of serialized PE time
    per duplicate."""
    removed = 0
    for f in nc.m.functions:
        for bb in f.blocks:
            insts = list(bb.instructions)
            last_sig = None
            to_remove = []
            for inst in insts:
                tn = type(inst).__name__
                if tn == "InstLdweights":
                    si = inst.sync_info
                    has_sync = si is not None and (
                        list(si.on_wait) or list(si.on_update)
                    )
                    sig = (
                        str(inst.ins[0]),
                        str(inst.tile_position),
                        str(inst.tile_size),
                        str(inst.perf_mode),
                        str(inst.is_transpose),
                    )
                    if sig == last_sig and not has_sync:
                        to_remove.append(inst)
                        continue
                    last_sig = sig
                elif tn == "InstMatmult":
                    continue
                elif getattr(inst, "engine", None) == mybir.EngineType.PE:
                    last_sig = None
            for inst in to_remove:
                bb.instructions.remove(inst)
                removed += 1
    return removed


_NC_CACHE = None


def build():
    global _NC_CACHE
    if _NC_CACHE is not None:
        return _NC_CACHE
    nc = bacc.Bacc("TRN2", target_bir_lowering=False, debug=False, num_devices=8)
    aps = {
        "xt": nc.dram_tensor("xt", [C, T], BF, kind="ExternalInput").ap(),
        "wq": nc.dram_tensor("wq", [C, 2 * P], BF, kind="ExternalInput").ap(),
        "wk": nc.dram_tensor("wk", [C, 2 * P], BF, kind="ExternalInput").ap(),
        "wv": nc.dram_tensor("wv", [C, 2 * P], BF, kind="ExternalInput").ap(),
        "bq": nc.dram_tensor("bq", [2 * P], F32, kind="ExternalInput").ap(),
        "bk": nc.dram_tensor("bk", [2 * P], F32, kind="ExternalInput").ap(),
        "wp": nc.dram_tensor("wp", [2 * P, C], BF, kind="ExternalInput").ap(),
        "mask": nc.dram_tensor("mask", [P, P], F32, kind="ExternalInput").ap(),
        "out": nc.dram_tensor("out", [T, C], F32, kind="ExternalOutput").ap(),
    }
    with tile.TileContext(nc) as tc:
        with ExitStack() as ctx:
            _emit(ctx, tc, aps)
    _dedupe_ldweights(nc)
    nc.compile()
    _NC_CACHE = nc
    return nc


def make_in_maps(x, Wqkv, bqkv, Wproj):
    """Host-side sharding/layout prep. Returns per-core input dicts."""
    bf = ml_dtypes.bfloat16
    scale = np.float32(1.0 / np.sqrt(DK))
    maskv = np.where(
        np.arange(P)[None, :] >= np.arange(P)[:, None], 0.0, NEG
    ).astype(np.float32)
    xts = [np.ascontiguousarray(x[b].T).astype(bf) for b in range(B)]
    in_maps = []
    for c in range(8):
        b, hg = divmod(c, 4)
        lo = hg * HPC * DK
        sl = slice(lo, lo + HPC * DK)
        in_maps.append(
            {
                "xt": xts[b],
                "wq": np.ascontiguousarray(Wqkv[:, 0 * C :][:, sl] * scale).astype(bf),
                "wk": np.ascontiguousarray(Wqkv[:, 1 * C :][:, sl]).astype(bf),
                "wv": np.ascontiguousarray(Wqkv[:, 2 * C :][:, sl]).astype(bf),
                "bq": np.ascontiguousarray(bqkv[0 * C :][sl] * scale).astype(np.float32),
                "bk": np.ascontiguousarray(bqkv[1 * C :][sl]).astype(np.float32),
                "wp": np.ascontiguousarray(Wproj[sl, :]).astype(bf),
                "mask": maskv,
            }
        )
    return in_maps


def gather(outs, bqkv, Wproj, bproj):
    """Sum per-core partials per batch; fold V-bias + proj-bias analytically."""
    bv = bqkv[2 * C :].astype(np.float32)
    bp_eff = (bproj.astype(np.float32) + bv @ Wproj.astype(np.float32)).astype(
        np.float32
    )
    y = np.empty((B, T, C), np.float32)
    for b in range(B):
        acc = outs[b * 4 + 0].astype(np.float32).copy()
        for hg in range(1, 4):
            acc += outs[b * 4 + hg]
        y[b] = acc + bp_eff[None, :]
    return y


def kernel(x, Wqkv, bqkv, Wproj, bproj):
    global LAST_RESULTS
    x = np.asarray(x, dtype=np.float32)
    Wqkv = np.asarray(Wqkv, dtype=np.float32)
    bqkv = np.asarray(bqkv, dtype=np.float32)
    Wproj = np.asarray(Wproj, dtype=np.float32)
    bproj = np.asarray(bproj, dtype=np.float32)

    nc = build()
    in_maps = make_in_maps(x, Wqkv, bqkv, Wproj)
    try:
        res = bass_utils.run_bass_kernel_spmd(
            nc,
            in_maps,
            core_ids=list(range(8)),
            trace=TRACE,
            **TRACE_KWARGS,
        )
    except Exception:
        if not TRACE:
            raise
        import traceback

        traceback.print_exc()
        print("traced run failed; retrying without trace", file=sys.stderr)
        res = bass_utils.run_bass_kernel_spmd(nc, in_maps, core_ids=list(range(8)))
    LAST_RESULTS = res
    outs = [res.results[c]["out"] for c in range(8)]
    return gather(outs, bqkv, Wproj, bproj)


# revision 34
# speedup vs baseline: 1.0080x; 1.0011x over previous
"""Causal self-attention (B=2, T=2048, C=1024, H=16) on 8 TRN2 NeuronCores.

Sharding: core c -> batch b = c//4, head group hg = c%4 (4 heads/core).
Each core computes QKV for its 4 heads (column-parallel), causal attention,
and a row-parallel partial output projection [T, C]. The host sums the 4
partials per batch and adds the analytically-folded biases.

Device layouts (chosen so no on-chip transposes are ever needed):
  xt   [C=1024, T=2048] bf16   x[b] transposed (host-prepped)
  Q^T  [128, pair, T]   bf16   head pair packed on partitions (0-63 / 64-127)
  K^T  same
  vaug [128, tj, 4*66]  bf16   per head: col0 = ones, cols1-64 = V[tj block]
  S^T  [k=128, q<=512]  psum   row-packed K=64 matmuls, 2 heads concurrent
  P^T = exp(S^T)        bf16   (no max subtraction; scores are ~N(0,1))
  O^T  [65, 512] psum:  row0 = softmax denominator l, rows 1-64 = (P@V)^T
  yt   [128(h,d), T]    bf16   normalized attention output, feeds proj lhsT
"""

import sys

if "/opt/trn_rl_repo" not in sys.path:
    sys.path.insert(0, "/opt/trn_rl_repo")

import numpy as np
import ml_dtypes
from contextlib import ExitStack

import concourse.bass as bass
import concourse.mybir as mybir
import concourse.tile as tile
from concourse import bacc, bass_utils
from concourse.bass import ds, ts



BF = mybir.dt.bfloat16
F32 = mybir.dt.float32

B, T, C = 2, 2048, 1024
H, DK = 16, 64
P = 128
KC = C // P          # 8 contraction chunks over C
NTG = T // 512       # 4 t-groups of 512
NTJ = T // 128       # 16 t-chunks of 128
HPC = 4              # heads per core
VS = 66              # vaug per-head stride (col0 ones, 1-64 V, 65 pad)

NEG = -30000.0

# module-level knobs for test harness
TRACE = False
TRACE_KWARGS = {}
LAST_RESULTS = None


def _emit(ctx, tc, aps):
    nc = tc.nc
    xt, wq, wk, wv, bq, bk, wp, mask, out = (
        aps["xt"], aps["wq"], aps["wk"], aps["wv"], aps["bq"], aps["bk"],
        aps["wp"], aps["mask"], aps["out"],
    )

    consts = ctx.enter_context(tc.tile_pool(name="consts", bufs=1))
    bigs = ctx.enter_context(tc.tile_pool(name="bigs", bufs=1))
    temps = ctx.enter_context(tc.tile_pool(name="temps", bufs=4))
    ppool = ctx.enter_context(tc.tile_pool(name="ppool", bufs=4))
    psum = ctx.enter_context(tc.tile_pool(name="psum", bufs=1, space="PSUM"))
    dpool = ctx.enter_context(tc.tile_pool(name="dpool", bufs=2, space="DRAM"))

    # ---- load inputs to SBUF (weights first — the first matmuls need them;
    # xts split per t-group, alternating DMA engines to parallelize) ----
    wqs = consts.tile([P, KC, 2 * P], BF)
    wqr = wq.rearrange("(k p) n -> p k n", p=P)
    nc.sync.dma_start(out=wqs[:, 0:2], in_=wqr[:, 0:2])
    nc.sync.dma_start(out=wqs[:, 2:], in_=wqr[:, 2:])
    wks = consts.tile([P, KC, 2 * P], BF)
    nc.gpsimd.dma_start(out=wks, in_=wk.rearrange("(k p) n -> p k n", p=P))
    bqs = consts.tile([P, 2], F32)
    nc.sync.dma_start(out=bqs, in_=bq.rearrange("(m p) -> p m", p=P))
    bks = consts.tile([P, 2], F32)
    nc.sync.dma_start(out=bks, in_=bk.rearrange("(m p) -> p m", p=P))
    maskt = consts.tile([P, P], F32)
    nc.sync.dma_start(out=maskt, in_=mask)

    xts = bigs.tile([P, KC, T], BF)
    xtr = xt.rearrange("(k p) t -> p k t", p=P)
    # first t-group split by k so the first QKV accumulation can begin
    # as soon as its k-chunks land
    for k0, k1 in ((0, 2), (2, 5), (5, 8)):
        nc.sync.dma_start(
            out=xts[:, k0:k1, 0:512], in_=xtr[:, k0:k1, 0:512]
        )
    for tg in range(1, NTG):
        eng = nc.sync if tg % 2 == 0 else nc.gpsimd
        eng.dma_start(out=xts[:, :, ts(tg, 512)], in_=xtr[:, :, ts(tg, 512)])

    wvs = consts.tile([P, KC, 2 * P], BF)
    nc.gpsimd.dma_start(out=wvs, in_=wv.rearrange("(k p) n -> p k n", p=P))
    wps = consts.tile([P, 2, C], BF)
    nc.sync.dma_start(out=wps, in_=wp.rearrange("(k p) n -> p k n", p=P))

    # ---- Q^T / K^T: [128(d pair-packed), pair, T] ----
    qt = bigs.tile([P, 2, T], BF)
    kt = bigs.tile([P, 2, T], BF)

    def emit_qk_pair(m, pg):
        # one LDWEIGHTS per (k, dst) feeds two N=512 matmuls (t-groups
        # 2*pg and 2*pg+1) — the second, identical LDW is deduped later
        tga, tgb = 2 * pg, 2 * pg + 1
        for wsrc, bsrc, dst in ((wqs, bqs, qt), (wks, bks, kt)):
            pq2 = [
                psum.tile([P, 512], F32, tag="mm", bufs=2, name=f"pq{i}")
                for i in range(2)
            ]
            for k in range(KC):
                for i, tg in enumerate((tga, tgb)):
                    nc.tensor.matmul(
                        pq2[i],
                        lhsT=wsrc[:, k, ts(m, P)],
                        rhs=xts[:, k, ts(tg, 512)],
                        start=(k == 0),
                        stop=(k == KC - 1),
                    )
            for i, tg in enumerate((tga, tgb)):
                nc.vector.tensor_add(
                    out=dst[:, m, ts(tg, 512)],
                    in0=pq2[i],
                    in1=bsrc[:, m : m + 1].to_broadcast([P, 512]),
                )

    # ---- V -> vaug [128, tj, 4*66] (col DK = ones) ----
    vaug = bigs.tile([P, NTJ, HPC * VS], BF)
    vaug4 = vaug.rearrange("p t (h c) -> p t h c", c=VS)

    def emit_v(g):
        for tj in range(4 * g, 4 * g + 4):
            pv = psum.tile([P, 512], F32, tag="mm", bufs=2, name="pv")
            for k in range(KC):
                nc.tensor.matmul(
                    pv[:, : 2 * P],
                    lhsT=xts[:, k, ts(tj, P)],
                    rhs=wvs[:, k, :],
                    start=(k == 0),
                    stop=(k == KC - 1),
                )
            nc.vector.tensor_copy(
                out=vaug4[:, tj, :, 0:DK],
                in_=pv[:, : 2 * P].rearrange("p (h d) -> p h d", d=DK),
            )

    # ---- attention ----
    yts = [bigs.tile([P, T], BF, name=f"yt{m}") for m in range(2)]

    def emit_attn(m, g):
        po = [
            psum.tile([DK + 1, 512], F32, tag=f"o{h}", bufs=1, name=f"po{h}")
            for h in range(2)
        ]
        njc = 4 * g + 4
        for j in range(njc):
            jrel = j - 4 * g
            band = jrel >= 0
            ncols = 512 - 128 * jrel if band else 512
            qoff = g * 512 + (128 * jrel if band else 0)
            pss = []
            for h in range(2):
                ps = psum.tile([P, 512], F32, tag=f"s{h}", bufs=2, name=f"ps{h}")
                nc.tensor.matmul(
                    ps[:, :ncols],
                    lhsT=kt[h * DK : (h + 1) * DK, m, ts(j, P)],
                    rhs=qt[h * DK : (h + 1) * DK, m, ds(qoff, ncols)],
                    start=True,
                    stop=True,
                    tile_position=(h * DK, 0),
                )
                pss.append(ps)
            if band:
                for h in range(2):
                    nc.vector.tensor_add(
                        out=pss[h][:, :P], in0=pss[h][:, :P], in1=maskt
                    )
            for h in range(2):
                pt = ppool.tile([P, 512], BF, tag=f"p{h}", name=f"pt{h}")
                nc.scalar.activation(
                    pt[:, :ncols],
                    pss[h][:, :ncols],
                    mybir.ActivationFunctionType.Exp,
                )
                co = 128 * jrel if band else 0
                nc.tensor.matmul(
                    po[h][:, co : co + ncols],
                    lhsT=vaug4[:, j, 2 * m + h, : DK + 1],
                    rhs=pt[:, :ncols],
                    start=(j == 0),
                    stop=(j == njc - 1),
                    skip_group_check=True,
                )
        # finalize: copy O^T off PSUM fast, then normalize rows 0-63 by the
        # broadcast exp-sum (row 64) and place into yt
        for h in range(2):
            oc = temps.tile([P, 512], F32, tag="oc", name="oc")
            nc.vector.tensor_copy(out=oc[: DK + 1, :], in_=po[h])
            dscr = dpool.tile([512], F32, tag="dscr", name="dscr")
            nc.sync.dma_start(out=dscr, in_=oc[DK : DK + 1, :])
            rbl = temps.tile([P, 512], F32, tag="rbl", name="rbl")
            nc.gpsimd.dma_start(
                out=rbl[:DK, :],
                in_=bass.AP(
                    tensor=dscr.tensor,
                    offset=dscr.offset,
                    ap=[[0, DK]] + list(dscr.ap),
                ),
            )
            rb = temps.tile([P, 512], F32, tag="rb", name="rb")
            nc.vector.reciprocal_approx_fast(out=rb[:DK, :], in_=rbl[:DK, :])
            stg = temps.tile([P, 512], BF, tag="stg", name="stg")
            nc.vector.tensor_mul(
                out=stg[:DK, :],
                in0=oc[:DK, :],
                in1=rb[:DK, :],
            )
            nc.sync.dma_start(
                out=yts[m][h * DK : (h + 1) * DK, ts(g, 512)],
                in_=stg[:DK, :],
            )

    # ---- output projection: partial [T, C] for one t-group of 4 chunks ----
    def emit_proj(g):
        for tj in range(4 * g, 4 * g + 4):
            pps = [
                psum.tile([P, 512], F32, tag="mm", bufs=2, name=f"pp{n}")
                for n in range(2)
            ]
            for kc in range(2):
                for n in range(2):
                    nc.tensor.matmul(
                        pps[n],
                        lhsT=yts[kc][:, ts(tj, P)],
                        rhs=wps[:, kc, ts(n, 512)],
                        start=(kc == 0),
                        stop=(kc == 1),
                    )
            for n in range(2):
                ostg = temps.tile([P, 512], F32, tag="ostg", name="ostg")
                nc.vector.tensor_copy(out=ostg, in_=pps[n])
                nc.sync.dma_start(out=out[ts(tj, P), ts(n, 512)], in_=ostg)

    # ---- schedule: pipeline by q-group, weaving PE-dense QKV/proj work
    # between ACT-gated attention so both engine queues stay fed. Group
    # order [1,2,3,0] puts the smallest attention group (g=0) last so the
    # kernel tail is short. QKV/V tiles are emitted incrementally just
    # before the first group that needs them. ----
    nc.vector.memset(vaug4[:, :, :, DK : DK + 1], 1.0)
    order = [1, 2, 3, 0]
    qk_done = [0, 0]  # per head-pair: number of t-group PAIRS emitted
    v_done = 0
    proj_queue = []
    for g in order:
        need_pg = g // 2 + 1
        need_v = g + 1
        while qk_done[0] < need_pg:
            emit_qk_pair(0, qk_done[0])
            qk_done[0] += 1
        while v_done < need_v:
            emit_v(v_done)
            v_done += 1
        emit_attn(0, g)
        while qk_done[1] < need_pg:
            emit_qk_pair(1, qk_done[1])
            qk_done[1] += 1
        emit_attn(1, g)
        proj_queue.append(g)
        if len(proj_queue) > 1:
            emit_proj(proj_queue.pop(0))
    for g in proj_queue:
        emit_proj(g)


def _dedupe_ldweights(nc):
    """Drop an InstLdweights when the immediately-preceding PE weight load in
    the scheduled stream is byte-identical (only matmuls in between — they
    don't disturb the stationary operand). Saves ~100ns of serialized PE time
    per duplicate."""
    removed = 0
    for f in nc.m.functions:
        for bb in f.blocks:
            insts = list(bb.instructions)
            last_sig = None
            to_remove = []
            for inst in insts:
                tn = type(inst).__name__
                if tn == "InstLdweights":
                    si = inst.sync_info
                    has_sync = si is not None and (
                        list(si.on_wait) or list(si.on_update)
                    )
                    sig = (
                        str(inst.ins[0]),
                        str(inst.tile_position),
                        str(inst.tile_size),
                        str(inst.perf_mode),
                        str(inst.is_transpose),
                    )
                    if sig == last_sig and not has_sync:
                        to_remove.append(inst)
                        continue
                    last_sig = sig
                elif tn == "InstMatmult":
                    continue
                elif getattr(inst, "engine", None) == mybir.EngineType.PE:
                    last_sig = None
            for inst in to_remove:
                bb.instructions.remove(inst)
                removed += 1
    return removed


_NC_CACHE = None


def build():
    global _NC_CACHE
    if _NC_CACHE is not None:
        return _NC_CACHE
    nc = bacc.Bacc("TRN2", target_bir_lowering=False, debug=False, num_devices=8)
    aps = {
        "xt": nc.dram_tensor("xt", [C, T], BF, kind="ExternalInput").ap(),
        "wq": nc.dram_tensor("wq", [C, 2 * P], BF, kind="ExternalInput").ap(),
        "wk": nc.dram_tensor("wk", [C, 2 * P], BF, kind="ExternalInput").ap(),
        "wv": nc.dram_tensor("wv", [C, 2 * P], BF, kind="ExternalInput").ap(),
        "bq": nc.dram_tensor("bq", [2 * P], F32, kind="ExternalInput").ap(),
        "bk": nc.dram_tensor("bk", [2 * P], F32, kind="ExternalInput").ap(),
        "wp": nc.dram_tensor("wp", [2 * P, C], BF, kind="ExternalInput").ap(),
        "mask": nc.dram_tensor("mask", [P, P], F32, kind="ExternalInput").ap(),
        "out": nc.dram_tensor("out", [T, C], F32, kind="ExternalOutput").ap(),
    }
    with tile.TileContext(nc) as tc:
        with ExitStack() as ctx:
            _emit(ctx, tc, aps)
    _dedupe_ldweights(nc)
    nc.compile()
    _NC_CACHE = nc
    return nc


def make_in_maps(x, Wqkv, bqkv, Wproj):
    """Host-side sharding/layout prep. Returns per-core input dicts."""
    bf = ml_dtypes.bfloat16
    scale = np.float32(1.0 / np.sqrt(DK))
    maskv = np.where(
        np.arange(P)[None, :] >= np.arange(P)[:, None], 0.0, NEG
    ).astype(np.float32)
    xts = [np.ascontiguousarray(x[b].T).astype(bf) for b in range(B)]
    in_maps = []
    for c in range(8):
        b, hg = divmod(c, 4)
        lo = hg * HPC * DK
        sl = slice(lo, lo + HPC * DK)
        in_maps.append(
            {
                "xt": xts[b],
                "wq": np.ascontiguousarray(Wqkv[:, 0 * C :][:, sl] * scale).astype(bf),
                "wk": np.ascontiguousarray(Wqkv[:, 1 * C :][:, sl]).astype(bf),
                "wv": np.ascontiguousarray(Wqkv[:, 2 * C :][:, sl]).astype(bf),
                "bq": np.ascontiguousarray(bqkv[0 * C :][sl] * scale).astype(np.float32),
                "bk": np.ascontiguousarray(bqkv[1 * C :][sl]).astype(np.float32),
                "wp": np.ascontiguousarray(Wproj[sl, :]).astype(bf),
                "mask": maskv,
            }
        )
    return in_maps


def gather(outs, bqkv, Wproj, bproj):
    """Sum per-core partials per batch; fold V-bias + proj-bias analytically."""
    bv = bqkv[2 * C :].astype(np.float32)
    bp_eff = (bproj.astype(np.float32) + bv @ Wproj.astype(np.float32)).astype(
        np.float32
    )
    y = np.empty((B, T, C), np.float32)
    for b in range(B):
        acc = outs[b * 4 + 0].astype(np.float32).copy()
        for hg in range(1, 4):
            acc += outs[b * 4 + hg]
        y[b] = acc + bp_eff[None, :]
    return y


def kernel(x, Wqkv, bqkv, Wproj, bproj):
    global LAST_RESULTS
    x = np.asarray(x, dtype=np.float32)
    Wqkv = np.asarray(Wqkv, dtype=np.float32)
    bqkv = np.asarray(bqkv, dtype=np.float32)
    Wproj = np.asarray(Wproj, dtype=np.float32)
    bproj = np.asarray(bproj, dtype=np.float32)

    nc = build()
    in_maps = make_in_maps(x, Wqkv, bqkv, Wproj)
    try:
        res = bass_utils.run_bass_kernel_spmd(
            nc,
            in_maps,
            core_ids=list(range(8)),
            trace=TRACE,
            **TRACE_KWARGS,
        )
    except Exception:
        if not TRACE:
            raise
        import traceback

        traceback.print_exc()
        print("traced run failed; retrying without trace", file=sys.stderr)
        res = bass_utils.run_bass_kernel_spmd(nc, in_maps, core_ids=list(range(8)))
    LAST_RESULTS = res
    outs = [res.results[c]["out"] for c in range(8)]
    return gather(outs, bqkv, Wproj, bproj)


# revision 38
# speedup vs baseline: 1.0083x; 1.0003x over previous
"""Causal self-attention (B=2, T=2048, C=1024, H=16) on 8 TRN2 NeuronCores.

Sharding: core c -> batch b = c//4, head group hg = c%4 (4 heads/core).
Each core computes QKV for its 4 heads (column-parallel), causal attention,
and a row-parallel partial output projection [T, C]. The host sums the 4
partials per batch and adds the analytically-folded biases.

Device layouts (chosen so no on-chip transposes are ever needed):
  xt   [C=1024, T=2048] bf16   x[b] transposed (host-prepped)
  Q^T  [128, pair, T]   bf16   head pair packed on partitions (0-63 / 64-127)
  K^T  same
  vaug [128, tj, 4*66]  bf16   per head: col0 = ones, cols1-64 = V[tj block]
  S^T  [k=128, q<=512]  psum   row-packed K=64 matmuls, 2 heads concurrent
  P^T = exp(S^T)        bf16   (no max subtraction; scores are ~N(0,1))
  O^T  [65, 512] psum:  row0 = softmax denominator l, rows 1-64 = (P@V)^T
  yt   [128(h,d), T]    bf16   normalized attention output, feeds proj lhsT
"""

import sys

if "/opt/trn_rl_repo" not in sys.path:
    sys.path.insert(0, "/opt/trn_rl_repo")

import numpy as np
import ml_dtypes
from contextlib import ExitStack

import concourse.bass as bass
import concourse.mybir as mybir
import concourse.tile as tile
from concourse import bacc, bass_utils
from concourse.bass import ds, ts



BF = mybir.dt.bfloat16
F32 = mybir.dt.float32

B, T, C = 2, 2048, 1024
H, DK = 16, 64
P = 128
KC = C // P          # 8 contraction chunks over C
NTG = T // 512       # 4 t-groups of 512
NTJ = T // 128       # 16 t-chunks of 128
HPC = 4              # heads per core
VS = 66              # vaug per-head stride (col0 ones, 1-64 V, 65 pad)

NEG = -30000.0

# module-level knobs for test harness
TRACE = False
TRACE_KWARGS = {}
LAST_RESULTS = None


def _emit(ctx, tc, aps):
    nc = tc.nc
    xt, wq, wk, wv, bq, bk, wp, mask, out = (
        aps["xt"], aps["wq"], aps["wk"], aps["wv"], aps["bq"], aps["bk"],
        aps["wp"], aps["mask"], aps["out"],
    )

    consts = ctx.enter_context(tc.tile_pool(name="consts", bufs=1))
    bigs = ctx.enter_context(tc.tile_pool(name="bigs", bufs=1))
    temps = ctx.enter_context(tc.tile_pool(name="temps", bufs=4))
    ppool = ctx.enter_context(tc.tile_pool(name="ppool", bufs=4))
    psum = ctx.enter_context(tc.tile_pool(name="psum", bufs=1, space="PSUM"))
    dpool = ctx.enter_context(tc.tile_pool(name="dpool", bufs=2, space="DRAM"))

    # ---- load inputs to SBUF. All DRAM inputs are pre-shaped on the host so
    # every DMA reads fully-linear DRAM (weights first — the first matmuls
    # need them; xt arrives as per-t-group blocks [tg][p][k][512]) ----
    wqs = consts.tile([P, KC, 2 * P], BF)
    nc.sync.dma_start(out=wqs, in_=wq)
    wks = consts.tile([P, KC, 2 * P], BF)
    nc.gpsimd.dma_start(out=wks, in_=wk)
    bqs = consts.tile([P, 2], F32)
    nc.sync.dma_start(out=bqs, in_=bq.rearrange("(m p) -> p m", p=P))
    bks = consts.tile([P, 2], F32)
    nc.sync.dma_start(out=bks, in_=bk.rearrange("(m p) -> p m", p=P))
    maskt = consts.tile([P, P], F32)
    nc.sync.dma_start(out=maskt, in_=mask)

    xts = bigs.tile([P, KC, T], BF)
    # first t-group split by k so the first QKV accumulation can begin
    # as soon as its k-chunks land
    for k0, k1 in ((0, 2), (2, 5), (5, 8)):
        nc.sync.dma_start(out=xts[:, k0:k1, 0:512], in_=xt[0, :, k0:k1, :])
    for tg in range(1, NTG):
        eng = nc.sync if tg % 2 == 0 else nc.gpsimd
        eng.dma_start(out=xts[:, :, ts(tg, 512)], in_=xt[tg])

    wvs = consts.tile([P, KC, 2 * P], BF)
    nc.gpsimd.dma_start(out=wvs, in_=wv)
    wps = consts.tile([P, 2, C], BF)
    nc.sync.dma_start(out=wps, in_=wp)

    # ---- Q^T / K^T: [128(d pair-packed), pair, T] ----
    qt = bigs.tile([P, 2, T], BF)
    kt = bigs.tile([P, 2, T], BF)

    def emit_qk_pair(m, pg):
        # one LDWEIGHTS per (k, dst) feeds two N=512 matmuls (t-groups
        # 2*pg and 2*pg+1) — the second, identical LDW is deduped later
        tga, tgb = 2 * pg, 2 * pg + 1
        for wsrc, bsrc, dst in ((wqs, bqs, qt), (wks, bks, kt)):
            pq2 = [
                psum.tile([P, 512], F32, tag="mm", bufs=2, name=f"pq{i}")
                for i in range(2)
            ]
            for k in range(KC):
                for i, tg in enumerate((tga, tgb)):
                    nc.tensor.matmul(
                        pq2[i],
                        lhsT=wsrc[:, k, ts(m, P)],
                        rhs=xts[:, k, ts(tg, 512)],
                        start=(k == 0),
                        stop=(k == KC - 1),
                    )
            for i, tg in enumerate((tga, tgb)):
                nc.vector.tensor_add(
                    out=dst[:, m, ts(tg, 512)],
                    in0=pq2[i],
                    in1=bsrc[:, m : m + 1].to_broadcast([P, 512]),
                )

    # ---- V -> vaug [128, tj, 4*66] (col DK = ones) ----
    vaug = bigs.tile([P, NTJ, HPC * VS], BF)
    vaug4 = vaug.rearrange("p t (h c) -> p t h c", c=VS)

    def emit_v(g):
        for tj in range(4 * g, 4 * g + 4):
            pv = psum.tile([P, 512], F32, tag="mm", bufs=2, name="pv")
            for k in range(KC):
                nc.tensor.matmul(
                    pv[:, : 2 * P],
                    lhsT=xts[:, k, ts(tj, P)],
                    rhs=wvs[:, k, :],
                    start=(k == 0),
                    stop=(k == KC - 1),
                )
            nc.vector.tensor_copy(
                out=vaug4[:, tj, :, 0:DK],
                in_=pv[:, : 2 * P].rearrange("p (h d) -> p h d", d=DK),
            )

    # ---- attention ----
    yts = [bigs.tile([P, T], BF, name=f"yt{m}") for m in range(2)]

    def emit_attn(m, g):
        po = [
            psum.tile([DK + 1, 512], F32, tag=f"o{h}", bufs=1, name=f"po{h}")
            for h in range(2)
        ]
        njc = 4 * g + 4
        for j in range(njc):
            jrel = j - 4 * g
            band = jrel >= 0
            ncols = 512 - 128 * jrel if band else 512
            qoff = g * 512 + (128 * jrel if band else 0)
            pss = []
            for h in range(2):
                ps = psum.tile([P, 512], F32, tag=f"s{h}", bufs=2, name=f"ps{h}")
                nc.tensor.matmul(
                    ps[:, :ncols],
                    lhsT=kt[h * DK : (h + 1) * DK, m, ts(j, P)],
                    rhs=qt[h * DK : (h + 1) * DK, m, ds(qoff, ncols)],
                    start=True,
                    stop=True,
                    tile_position=(h * DK, 0),
                )
                pss.append(ps)
            if band:
                for h in range(2):
                    nc.vector.tensor_add(
                        out=pss[h][:, :P], in0=pss[h][:, :P], in1=maskt
                    )
            for h in range(2):
                pt = ppool.tile([P, 512], BF, tag=f"p{h}", name=f"pt{h}")
                nc.scalar.activation(
                    pt[:, :ncols],
                    pss[h][:, :ncols],
                    mybir.ActivationFunctionType.Exp,
                )
                co = 128 * jrel if band else 0
                nc.tensor.matmul(
                    po[h][:, co : co + ncols],
                    lhsT=vaug4[:, j, 2 * m + h, : DK + 1],
                    rhs=pt[:, :ncols],
                    start=(j == 0),
                    stop=(j == njc - 1),
                    skip_group_check=True,
                )
        # finalize: copy O^T off PSUM fast, then normalize rows 0-63 by the
        # broadcast exp-sum (row 64) and place into yt
        for h in range(2):
            oc = temps.tile([P, 512], F32, tag="oc", name="oc")
            nc.vector.tensor_copy(out=oc[: DK + 1, :], in_=po[h])
            dscr = dpool.tile([512], F32, tag="dscr", name="dscr")
            nc.sync.dma_start(out=dscr, in_=oc[DK : DK + 1, :])
            rbl = temps.tile([P, 512], F32, tag="rbl", name="rbl")
            nc.gpsimd.dma_start(
                out=rbl[:DK, :],
                in_=bass.AP(
                    tensor=dscr.tensor,
                    offset=dscr.offset,
                    ap=[[0, DK]] + list(dscr.ap),
                ),
            )
            rb = temps.tile([P, 512], F32, tag="rb", name="rb")
            nc.vector.reciprocal_approx_fast(out=rb[:DK, :], in_=rbl[:DK, :])
            stg = temps.tile([P, 512], BF, tag="stg", name="stg")
            nc.vector.tensor_mul(
                out=stg[:DK, :],
                in0=oc[:DK, :],
                in1=rb[:DK, :],
            )
            nc.sync.dma_start(
                out=yts[m][h * DK : (h + 1) * DK, ts(g, 512)],
                in_=stg[:DK, :],
            )

    # ---- output projection: partial [T, C] for one t-group of 4 chunks ----
    def emit_proj(g):
        for tj in range(4 * g, 4 * g + 4):
            pps = [
                psum.tile([P, 512], F32, tag="mm", bufs=2, name=f"pp{n}")
                for n in range(2)
            ]
            for kc in range(2):
                for n in range(2):
                    nc.tensor.matmul(
                        pps[n],
                        lhsT=yts[kc][:, ts(tj, P)],
                        rhs=wps[:, kc, ts(n, 512)],
                        start=(kc == 0),
                        stop=(kc == 1),
                    )
            for n in range(2):
                ostg = temps.tile([P, 512], F32, tag="ostg", name="ostg")
                nc.vector.tensor_copy(out=ostg, in_=pps[n])
                nc.sync.dma_start(out=out[ts(tj, P), ts(n, 512)], in_=ostg)

    # ---- schedule: pipeline by q-group, weaving PE-dense QKV/proj work
    # between ACT-gated attention so both engine queues stay fed. Group
    # order [1,2,3,0] puts the smallest attention group (g=0) last so the
    # kernel tail is short. QKV/V tiles are emitted incrementally just
    # before the first group that needs them. ----
    nc.vector.memset(vaug4[:, :, :, DK : DK + 1], 1.0)
    order = [1, 2, 3, 0]
    qk_done = [0, 0]  # per head-pair: number of t-group PAIRS emitted
    v_done = 0
    proj_queue = []
    for g in order:
        need_pg = g // 2 + 1
        need_v = g + 1
        while qk_done[0] < need_pg:
            emit_qk_pair(0, qk_done[0])
            qk_done[0] += 1
        while v_done < need_v:
            emit_v(v_done)
            v_done += 1
        emit_attn(0, g)
        while qk_done[1] < need_pg:
            emit_qk_pair(1, qk_done[1])
            qk_done[1] += 1
        emit_attn(1, g)
        proj_queue.append(g)
        if len(proj_queue) > 1:
            emit_proj(proj_queue.pop(0))
    for g in proj_queue:
        emit_proj(g)


def _dedupe_ldweights(nc):
    """Drop an InstLdweights when the immediately-preceding PE weight load in
    the scheduled stream is byte-identical (only matmuls in between — they
    don't disturb the stationary operand). Saves ~100ns of serialized PE time
    per duplicate."""
    removed = 0
    for f in nc.m.functions:
        for bb in f.blocks:
            insts = list(bb.instructions)
            last_sig = None
            to_remove = []
            for inst in insts:
                tn = type(inst).__name__
                if tn == "InstLdweights":
                    si = inst.sync_info
                    has_sync = si is not None and (
                        list(si.on_wait) or list(si.on_update)
                    )
                    sig = (
                        str(inst.ins[0]),
                        str(inst.tile_position),
                        str(inst.tile_size),
                        str(inst.perf_mode),
                        str(inst.is_transpose),
                    )
                    if sig == last_sig and not has_sync:
                        to_remove.append(inst)
                        continue
                    last_sig = sig
                elif tn == "InstMatmult":
                    continue
                elif getattr(inst, "engine", None) == mybir.EngineType.PE:
                    last_sig = None
            for inst in to_remove:
                bb.instructions.remove(inst)
                removed += 1
    return removed


_NC_CACHE = None


def build():
    global _NC_CACHE
    if _NC_CACHE is not None:
        return _NC_CACHE
    nc = bacc.Bacc("TRN2", target_bir_lowering=False, debug=False, num_devices=8)
    aps = {
        "xt": nc.dram_tensor("xt", [NTG, P, KC, 512], BF, kind="ExternalInput").ap(),
        "wq": nc.dram_tensor("wq", [P, KC, 2 * P], BF, kind="ExternalInput").ap(),
        "wk": nc.dram_tensor("wk", [P, KC, 2 * P], BF, kind="ExternalInput").ap(),
        "wv": nc.dram_tensor("wv", [P, KC, 2 * P], BF, kind="ExternalInput").ap(),
        "bq": nc.dram_tensor("bq", [2 * P], F32, kind="ExternalInput").ap(),
        "bk": nc.dram_tensor("bk", [2 * P], F32, kind="ExternalInput").ap(),
        "wp": nc.dram_tensor("wp", [P, 2, C], BF, kind="ExternalInput").ap(),
        "mask": nc.dram_tensor("mask", [P, P], F32, kind="ExternalInput").ap(),
        "out": nc.dram_tensor("out", [T, C], F32, kind="ExternalOutput").ap(),
    }
    with tile.TileContext(nc) as tc:
        with ExitStack() as ctx:
            _emit(ctx, tc, aps)
    _dedupe_ldweights(nc)
    nc.compile()
    _NC_CACHE = nc
    return nc


def make_in_maps(x, Wqkv, bqkv, Wproj):
    """Host-side sharding/layout prep. Returns per-core input dicts."""
    bf = ml_dtypes.bfloat16
    scale = np.float32(1.0 / np.sqrt(DK))
    maskv = np.where(
        np.arange(P)[None, :] >= np.arange(P)[:, None], 0.0, NEG
    ).astype(np.float32)
    def lay_w(w):  # [C, n] -> [p, k, n] linear
        n = w.shape[1]
        return np.ascontiguousarray(
            w.reshape(KC, P, n).transpose(1, 0, 2)
        ).astype(bf)

    def lay_x(xb):  # [T, C] -> [tg, p, k, 512] linear
        xt = xb.T  # [C, T]
        return np.ascontiguousarray(
            xt.reshape(KC, P, NTG, 512).transpose(2, 1, 0, 3)
        ).astype(bf)

    xts = [lay_x(x[b]) for b in range(B)]
    in_maps = []
    for c in range(8):
        b, hg = divmod(c, 4)
        lo = hg * HPC * DK
        sl = slice(lo, lo + HPC * DK)
        in_maps.append(
            {
                "xt": xts[b],
                "wq": lay_w(Wqkv[:, 0 * C :][:, sl] * scale),
                "wk": lay_w(Wqkv[:, 1 * C :][:, sl]),
                "wv": lay_w(Wqkv[:, 2 * C :][:, sl]),
                "bq": np.ascontiguousarray(bqkv[0 * C :][sl] * scale).astype(np.float32),
                "bk": np.ascontiguousarray(bqkv[1 * C :][sl]).astype(np.float32),
                "wp": np.ascontiguousarray(
                    Wproj[sl, :].reshape(2, P, C).transpose(1, 0, 2)
                ).astype(bf),
                "mask": maskv,
            }
        )
    return in_maps


def gather(outs, bqkv, Wproj, bproj):
    """Sum per-core partials per batch; fold V-bias + proj-bias analytically."""
    bv = bqkv[2 * C :].astype(np.float32)
    bp_eff = (bproj.astype(np.float32) + bv @ Wproj.astype(np.float32)).astype(
        np.float32
    )
    y = np.empty((B, T, C), np.float32)
    for b in range(B):
        acc = outs[b * 4 + 0].astype(np.float32).copy()
        for hg in range(1, 4):
            acc += outs[b * 4 + hg]
        y[b] = acc + bp_eff[None, :]
    return y


def kernel(x, Wqkv, bqkv, Wproj, bproj):
    global LAST_RESULTS
    x = np.asarray(x, dtype=np.float32)
    Wqkv = np.asarray(Wqkv, dtype=np.float32)
    bqkv = np.asarray(bqkv, dtype=np.float32)
    Wproj = np.asarray(Wproj, dtype=np.float32)
    bproj = np.asarray(bproj, dtype=np.float32)

    nc = build()
    in_maps = make_in_maps(x, Wqkv, bqkv, Wproj)
    try:
        res = bass_utils.run_bass_kernel_spmd(
            nc,
            in_maps,
            core_ids=list(range(8)),
            trace=TRACE,
            **TRACE_KWARGS,
        )
    except Exception:
        if not TRACE:
            raise
        import traceback

        traceback.print_exc()
        print("traced run failed; retrying without trace", file=sys.stderr)
        res = bass_utils.run_bass_kernel_spmd(nc, in_maps, core_ids=list(range(8)))
    LAST_RESULTS = res
    outs = [res.results[c]["out"] for c in range(8)]
    return gather(outs, bqkv, Wproj, bproj)
